# revision 13
# baseline (speedup 1.0000x reference)
"""Trainium2 Bass kernel for nn_MCModel_84559316123793.

The reference iterates w <- A @ w idx_T times (tridiagonal transition
matrix, absorbing boundaries) and returns (A^T)[IDX_Z, idx_s].  With the
start index interior, the dynamics live in the (NX-1)-dim tridiagonal
Toeplitz matrix B = tridiag(p2, pmid, p1) whose eigensystem is the
discrete sine transform:

  (B^T)[z,s] = (2/NX) * (p2/p1)^((z-s)/2)
               * sum_k lam_k^T sin(z k pi/NX) sin(s k pi/NX),
  lam_k = pmid + 2 sqrt(p1 p2) cos(k pi/NX).

z = 512 makes sin(z k pi/NX) vanish for even k, so only odd modes carry
weight, and for T >~ 1000 only the few dozen lowest survive exp
underflow; the host picks the survivors (mu-independently, from T alone)
and shards them across the 8 cores.

All mu-dependence reduces, to O(u^2) accuracy (u = (mu*DT/DX)^2 <~ 3e-5),
to two scalars the device derives from the DMA'd mu:

  u = (mu*C2)^2,   v = mu*LNPX2        (e*ln(p2/p1) linearised in mu)
  pe_k = v - P0_k - u*P1_k             (host tables: P0 = -T ln lam_k|_0,
                                        P1 = dP0/du, exact at u=0)
  out  = sum_k w_k exp(pe_k)           (w_k = signed DST weights)

Five engine instructions total (3 DVE + 1 ACT exp + 1 DVE reduce)
between an input DMA and an output DMA, in a raw-Bass Block (no Tile
prologue/epilogue).  The host sums the 8 per-core partials and applies
the mu-independent prefactor 2/NX.  Linearisation error is <~ 5e-5
relative for |mu| <= 5 (tolerance 2e-2).
"""

import math

import numpy as np

import concourse.bass as bass
import concourse.mybir as mybir
from concourse.bass_utils import run_bass_kernel_spmd

# Model constants (fixed by the problem definition)
SIGMA = 1.0
A_DOM = 2.0
Z_POS = 1.0
DT = 2e-06
NX = 1024
DX = A_DOM / NX
IDX_Z = int(round(Z_POS / DX))  # 512

N_CORES = 8

F32 = mybir.dt.float32
AF = mybir.ActivationFunctionType
ALU = mybir.AluOpType

# Derived immediates
A0 = SIGMA * SIGMA * DT / (DX * DX)  # p1+p2 at mu=0 (= 0.524288)
C2 = DT / DX                         # x = mu*C2 = p1-p2
A1 = (2.0 * A0 - 1.0) / (2.0 * A0)   # d(2 sqrt(p1 p2))/du
TC = 1.0 / (2.0 * A0)                # d(p1+p2-2 sqrt(p1 p2))/du

# Modes whose T*|ln lam| exceeds this are exp-underflow dead for any
# plausible mu shift (|u*P1| + |v| << 20) and are dropped host-side.
P0_CUT = 120.0
P0_DEAD = 200.0  # pe = v - 200 -> exp == 0.0f


def _split_multiwaits(nc):
    """This container's walrus rejects instructions carrying more than one
    sem-wait ("Too many sync wait commands").  Hoist all but the last onto
    single-wait NOPs inserted just before the offender on the same engine."""
    for bb in nc.main_func.blocks:
        insts = list(bb.instructions)
        changed = False
        out = []
        for ins in insts:
            si = ins.sync_info
            if si is not None and len(si.on_wait) > 1:
                waits = list(si.on_wait)
                for w in waits[:-1]:
                    nop = mybir.InstNoOp(
                        name=f"{ins.name}-wsplit-{w.ant_name}", ins=[], outs=[])
                    nop.engine = ins.engine
                    nop.sync_info = mybir.SyncInfo(on_wait=[w], on_update=[])
                    out.append(nop)
                ins.sync_info = mybir.SyncInfo(
                    on_wait=[waits[-1]], on_update=list(si.on_update))
                changed = True
            out.append(ins)
        if changed:
            bb.instructions = out


def _mode_tables(T: int, s_eff: int, extra_p2: bool):
    """Per-core host tables (mu-independent; they bake only T, s, geometry).

    Returns (npc, per-core [P0n | P1n | W] rows, device immediates, post
    scale).  P0n/P1n are negated so the device computes g = -P0 - u*P1 with
    a single (P1n * u) + P0n op and pe = g + v inside the ACT exp bias."""
    k = np.arange(1, NX, 2, dtype=np.float64)  # odd modes only (z = 512)
    th = k * np.pi / NX
    om = 1.0 - np.cos(th)
    lam0 = 1.0 - A0 * om                       # lam at u=0 (exact)
    P0 = -T * np.log(np.abs(lam0))
    P1 = T * (TC + A1 * om) / lam0             # dP0/du, exact at u=0
    w = np.sin(IDX_Z * th) * np.sin(s_eff * th)
    if T % 2 == 1:
        w = w * np.sign(lam0)

    # s==0 factor p2 = (A0 - x + u)/2: ln p2 folds into the linear model as
    # ln(A0/2) - x/A0 + u*(1/A0 - 1/(2 A0^2)); the constant goes to the
    # host post-scale, the x term into LNPX2, the u term into P1.
    e_coef = 0.5 * (IDX_Z - s_eff)
    lnp_x = -2.0 * e_coef / A0                 # d(e*ln(p2/p1))/dx
    post = 2.0 / NX
    if extra_p2:
        lnp_x -= 1.0 / A0
        P1 = P1 - (1.0 / A0 - 1.0 / (2.0 * A0 * A0))
        post *= A0 / 2.0

    keep = P0 <= P0_CUT
    nkeep = int(np.count_nonzero(keep))
    npc = min(len(k) // N_CORES, max(2, -(-nkeep // N_CORES)))
    P0k, P1k, wk = P0[keep][: npc * N_CORES], P1[keep][: npc * N_CORES], \
        w[keep][: npc * N_CORES]

    rows = []
    for c in range(N_CORES):
        sl = slice(c * npc, (c + 1) * npc)
        p0r = np.full(npc, P0_DEAD)
        p1r = np.zeros(npc)
        wr = np.zeros(npc)
        n = len(P0k[sl])
        p0r[:n], p1r[:n], wr[:n] = P0k[sl], P1k[sl], wk[sl]
        rows.append(np.concatenate([-p0r, -p1r, wr]).astype(np.float32))

    imm = {"LNPX2": C2 * lnp_x, "C2SQ": C2 * C2}
    return npc, rows, imm, post


def _build_program(T: int, s_eff: int, extra_p2: bool):
    """Emit the SPMD per-core program: DMA in -> 5 ops -> DMA out."""
    npc, rows, imm, post = _mode_tables(T, s_eff, extra_p2)
    nc = bass.Bass()

    xin = nc.declare_dram_parameter("xin", [1, 1 + 3 * npc], F32,
                                    isOutput=False)
    out = nc.declare_dram_parameter("out", [1, 1], F32, isOutput=True)

    with (
        nc.Block(no_gpsimd_drain=True) as block,
        nc.semaphore("dsem") as dsem,
        nc.semaphore("esem") as esem,
        nc.semaphore("asem") as asem,
        nc.semaphore("vsem") as vsem,
        nc.semaphore("osem") as osem,
        nc.semaphore("usem") as usem,
        nc.sbuf_tensor("x", [1, 1 + 3 * npc], F32) as x,
        nc.sbuf_tensor("v", [1, 1], F32) as v,
        nc.sbuf_tensor("u", [1, 1], F32) as u,
        nc.sbuf_tensor("g", [1, npc], F32) as g,
        nc.sbuf_tensor("pw", [1, npc], F32) as pw,
        nc.sbuf_tensor("tm", [1, npc], F32) as tm,
        nc.sbuf_tensor("acc", [1, 1], F32) as acc,
    ):
        mu = x[:, 0:1]
        p0n = x[:, 1:1 + npc]
        p1n = x[:, 1 + npc:1 + 2 * npc]
        wt = x[:, 1 + 2 * npc:1 + 3 * npc]

        # Semaphores are NOT zeroed on allocation, and earlier NEFFs on the
        # same device leave residue.  Each consumer clears the sems it waits
        # on at stream start; every producer's first inc is >2us later (the
        # input-DMA latency), so clear-before-inc holds by construction.
        @block.sync
        def _(sync):
            sync.sem_clear(vsem)
            sync.dma_start(x[:, :], xin[:, :]).then_inc(dsem, 16)
            # Output DMA from SP's HWDGE.  Walrus requires every DGE DMA to
            # carry a completion-sem update; the wait rides the DMA itself.
            sync.dma_start(out[:, :], acc[:, :])._wait_ge(vsem, 1).then_inc(
                osem, 16)

        @block.vector
        def _(vector):
            vector.sem_clear(dsem)
            vector.sem_clear(asem)
            vector.sem_clear(usem)
            vector.wait_ge(dsem, 16)
            # v = mu * LNPX2  (linearised e*ln(p2/p1) [+ ln p2 term])
            vector.tensor_scalar(v[:, :], mu, imm["LNPX2"], None,
                                 op0=ALU.mult)
            # u = (mu * mu) * C2^2.  The scalar-ptr operand of the next op
            # is fetched at dispatch, not engine-exec, so the in-order
            # engine does NOT cover the u hazard: round-trip a self-sem.
            vector.tensor_scalar(u[:, :], mu, mu, imm["C2SQ"],
                                 op0=ALU.mult, op1=ALU.mult).then_inc(usem, 1)
            vector.wait_ge(usem, 1)
            # g = (P1n * u) + P0n   == -(P0 + u*P1)
            vector.scalar_tensor_tensor(g[:, :], p1n, u[:, 0:1], p0n,
                                        op0=ALU.mult,
                                        op1=ALU.add).then_inc(esem, 1)

        @block.scalar
        def _(scalar):
            scalar.sem_clear(esem)
            scalar.wait_ge(esem, 1)
            # pw = exp(g + v)
            scalar.activation(pw[:, :], g[:, :], AF.Exp, bias=v[:, 0:1],
                              scale=1.0).then_inc(asem, 1)

        @block.vector
        def _(vector):
            vector.wait_ge(asem, 1)
            # acc = sum(pw * w)
            vector.scalar_tensor_tensor(tm[:, :], pw[:, :], 1.0, wt,
                                        op0=ALU.mult, op1=ALU.mult,
                                        accum_out=acc[:, :]).then_inc(vsem, 1)



    _split_multiwaits(nc)
    return nc, rows, post


def _in_maps(mu_val, rows):
    maps = []
    for r in rows:
        xin = np.empty((1, 1 + len(r)), dtype=np.float32)
        xin[0, 0] = mu_val
        xin[0, 1:] = r
        maps.append({"xin": xin})
    return maps


def build_program(T: int, s: int):
    """The program actually run/timed for inputs (T, s); handles the s==0
    remap.  Returns (nc, rows, post) or None if the answer is closed-form."""
    if T == 0:
        return None
    if s == 0:
        if T == 1:
            return None
        return _build_program(T - 1, 1, True)
    return _build_program(T, s, False)


def kernel(mu: np.ndarray, idx_T, idx_s) -> np.ndarray:
    T = int(idx_T)
    s = int(idx_s)
    mu_val = np.float32(np.asarray(mu).reshape(-1)[0])

    if T == 0:
        # A^0 = I
        return np.array([[1.0 if s == IDX_Z else 0.0]], dtype=np.float32)
    if s == 0 and T == 1:
        return np.array([[0.0]], dtype=np.float32)  # row IDX_Z sees nothing

    nc, rows, post = build_program(T, s)
    results = run_bass_kernel_spmd(nc, _in_maps(mu_val, rows),
                                   list(range(N_CORES))).results
    total = math.fsum(float(results[c]["out"][0, 0]) for c in range(N_CORES))
    return np.array([[total * post]], dtype=np.float32)


if __name__ == "__main__":
    out = kernel(np.array([-1.3152148], dtype=np.float32), 10000, 256)
    print("kernel output:", out)


# revision 18
# speedup vs baseline: 1.5374x; 1.5374x over previous
"""Trainium2 Bass kernel for nn_MCModel_84559316123793.

The reference iterates w <- A @ w idx_T times (tridiagonal transition
matrix, absorbing boundaries) and returns (A^T)[IDX_Z, idx_s].  With the
start index interior, the dynamics live in the (NX-1)-dim tridiagonal
Toeplitz matrix B = tridiag(p2, pmid, p1) whose eigensystem is the
discrete sine transform:

  (B^T)[z,s] = (2/NX) * (p2/p1)^((z-s)/2)
               * sum_k lam_k^T sin(z k pi/NX) sin(s k pi/NX),
  lam_k = pmid + 2 sqrt(p1 p2) cos(k pi/NX).

z = 512 makes sin(z k pi/NX) vanish for even k, so only odd modes carry
weight, and for T >~ 1000 only the few dozen lowest survive exp
underflow; the host picks the survivors (mu-independently, from T alone)
and shards them across the 8 cores.

All mu-dependence reduces, to O(u^2) accuracy (u = (mu*DT/DX)^2 <~ 3e-5),
to two scalars the device derives from the DMA'd mu:

  u = (mu*C2)^2,   v = mu*LNPX2        (e*ln(p2/p1) linearised in mu)
  pe_k = v - P0_k - u*P1_k             (host tables: P0 = -T ln lam_k|_0,
                                        P1 = dP0/du, exact at u=0)
  out  = sum_k w_k exp(pe_k)           (w_k = signed DST weights)

Five engine instructions total (3 DVE + 1 ACT exp + 1 DVE reduce)
between an input DMA and an output DMA, in a raw-Bass Block (no Tile
prologue/epilogue).  The host sums the 8 per-core partials and applies
the mu-independent prefactor 2/NX.  Linearisation error is <~ 5e-5
relative for |mu| <= 5 (tolerance 2e-2).
"""

import math

import numpy as np

import concourse.bass as bass
import concourse.mybir as mybir
from concourse import library_config
from concourse.bass_utils import run_bass_kernel_spmd

# Model constants (fixed by the problem definition)
SIGMA = 1.0
A_DOM = 2.0
Z_POS = 1.0
DT = 2e-06
NX = 1024
DX = A_DOM / NX
IDX_Z = int(round(Z_POS / DX))  # 512

N_CORES = 8

F32 = mybir.dt.float32
AF = mybir.ActivationFunctionType
ALU = mybir.AluOpType

# Derived immediates
A0 = SIGMA * SIGMA * DT / (DX * DX)  # p1+p2 at mu=0 (= 0.524288)
C2 = DT / DX                         # x = mu*C2 = p1-p2
A1 = (2.0 * A0 - 1.0) / (2.0 * A0)   # d(2 sqrt(p1 p2))/du
TC = 1.0 / (2.0 * A0)                # d(p1+p2-2 sqrt(p1 p2))/du

# Modes whose T*|ln lam| exceeds this are exp-underflow dead for any
# plausible mu shift (|u*P1| + |v| << 20) and are dropped host-side.
P0_CUT = 120.0
P0_DEAD = 200.0  # pe = v - 200 -> exp == 0.0f


def _split_multiwaits(nc):
    """This container's walrus rejects instructions carrying more than one
    sem-wait ("Too many sync wait commands").  Hoist all but the last onto
    single-wait NOPs inserted just before the offender on the same engine."""
    for bb in nc.main_func.blocks:
        insts = list(bb.instructions)
        changed = False
        out = []
        for ins in insts:
            si = ins.sync_info
            if si is not None and len(si.on_wait) > 1:
                waits = list(si.on_wait)
                for w in waits[:-1]:
                    nop = mybir.InstNoOp(
                        name=f"{ins.name}-wsplit-{w.ant_name}", ins=[], outs=[])
                    nop.engine = ins.engine
                    nop.sync_info = mybir.SyncInfo(on_wait=[w], on_update=[])
                    out.append(nop)
                ins.sync_info = mybir.SyncInfo(
                    on_wait=[waits[-1]], on_update=list(si.on_update))
                changed = True
            out.append(ins)
        if changed:
            bb.instructions = out


def _mode_tables(T: int, s_eff: int, extra_p2: bool):
    """Per-core host tables (mu-independent; they bake only T, s, geometry).

    Returns (npc, per-core [P0n | P1n | W] rows, device immediates, post
    scale).  P0n/P1n are negated so the device computes g = -P0 - u*P1 with
    a single (P1n * u) + P0n op and pe = g + v inside the ACT exp bias."""
    k = np.arange(1, NX, 2, dtype=np.float64)  # odd modes only (z = 512)
    th = k * np.pi / NX
    om = 1.0 - np.cos(th)
    lam0 = 1.0 - A0 * om                       # lam at u=0 (exact)
    P0 = -T * np.log(np.abs(lam0))
    P1 = T * (TC + A1 * om) / lam0             # dP0/du, exact at u=0
    w = np.sin(IDX_Z * th) * np.sin(s_eff * th)
    if T % 2 == 1:
        w = w * np.sign(lam0)

    # s==0 factor p2 = (A0 - x + u)/2: ln p2 folds into the linear model as
    # ln(A0/2) - x/A0 + u*(1/A0 - 1/(2 A0^2)); the constant goes to the
    # host post-scale, the x term into LNPX2, the u term into P1.
    e_coef = 0.5 * (IDX_Z - s_eff)
    lnp_x = -2.0 * e_coef / A0                 # d(e*ln(p2/p1))/dx
    post = 2.0 / NX
    if extra_p2:
        lnp_x -= 1.0 / A0
        P1 = P1 - (1.0 / A0 - 1.0 / (2.0 * A0 * A0))
        post *= A0 / 2.0

    keep = P0 <= P0_CUT
    nkeep = int(np.count_nonzero(keep))
    npc = min(len(k) // N_CORES, max(2, -(-nkeep // N_CORES)))
    P0k, P1k, wk = P0[keep][: npc * N_CORES], P1[keep][: npc * N_CORES], \
        w[keep][: npc * N_CORES]

    rows = []
    for c in range(N_CORES):
        sl = slice(c * npc, (c + 1) * npc)
        p0r = np.full(npc, P0_DEAD)
        p1r = np.zeros(npc)
        wr = np.zeros(npc)
        n = len(P0k[sl])
        p0r[:n], p1r[:n], wr[:n] = P0k[sl], P1k[sl], wk[sl]
        rows.append(np.concatenate([-p0r, -p1r, wr]).astype(np.float32))

    imm = {"LNPX2": C2 * lnp_x, "C2SQ": C2 * C2}
    return npc, rows, imm, post


def _strip_init_preamble(nc, pre_names):
    """Remove Bass.__init__'s const-AP memsets and its all-engine barrier
    (~930ns before the input DMA can issue).  This program never reads the
    const APs, and its own semaphores carry all cross-engine ordering."""
    kill = (mybir.InstMemset, mybir.InstDrain, mybir.InstEventSemaphore)
    for bb in nc.main_func.blocks:
        bb.instructions = [
            ins for ins in bb.instructions
            if not (ins.name in pre_names and isinstance(ins, kill))
        ]


def _build_program(T: int, s_eff: int, extra_p2: bool):
    """Emit the SPMD per-core program: DMA in -> 5 ops -> triggered DMA out.

    The output rides a kv_writeback descriptor prepared on the Pool engine
    during the input-DMA dead time; once the reduce lands, trigger_dma
    fires it, so the post-compute output cost is ~50ns + transfer + the
    mandatory DMA-sem propagation instead of a full HWDGE round trip."""
    npc, rows, imm, post = _mode_tables(T, s_eff, extra_p2)
    nc = bass.Bass()
    pre_names = {ins.name for bb in nc.main_func.blocks
                 for ins in bb.instructions}

    xin = nc.declare_dram_parameter("xin", [1, 1 + 3 * npc], F32,
                                    isOutput=False)
    # kv_writeback scatters one value per partition: out[0, p] <- col[p].
    # Only out[0, 0] (partition 0 = the reduce accumulator) is meaningful.
    out = nc.declare_dram_parameter("out", [1, 128], F32, isOutput=True)

    with (
        nc.Block(no_gpsimd_drain=True) as block,
        nc.semaphore("dsem") as dsem,
        nc.semaphore("esem") as esem,
        nc.semaphore("asem") as asem,
        nc.semaphore("vsem") as vsem,
        nc.semaphore("usem") as usem,
        nc.semaphore("psem") as psem,
        nc.semaphore("csem") as csem,
        nc.semaphore("odsem") as odsem,
        nc.sbuf_tensor("x", [1, 1 + 3 * npc], F32) as x,
        nc.sbuf_tensor("v", [1, 1], F32) as v,
        nc.sbuf_tensor("u", [1, 1], F32) as u,
        nc.sbuf_tensor("g", [1, npc], F32) as g,
        nc.sbuf_tensor("pw", [1, npc], F32) as pw,
        nc.sbuf_tensor("tm", [1, npc], F32) as tm,
        nc.sbuf_tensor("col", [128, 1], F32) as col,
        nc.sbuf_tensor("cidx", [128, 1], mybir.dt.int32) as cidx,
    ):
        mu = x[:, 0:1]
        p0n = x[:, 1:1 + npc]
        p1n = x[:, 1 + npc:1 + 2 * npc]
        wt = x[:, 1 + 2 * npc:1 + 3 * npc]
        acc = col[0:1, 0:1]

        # Semaphores are NOT zeroed on allocation, and earlier NEFFs on the
        # same device leave residue.  Each consumer clears the sems it waits
        # on at stream start; every producer's first inc is >2us later (the
        # input-DMA latency), so clear-before-inc holds by construction.
        @block.sync
        def _(sync):
            sync.dma_start(x[:, :], xin[:, :]).then_inc(dsem, 16)

        @block.vector
        def _(vector):
            vector.sem_clear(dsem)
            vector.sem_clear(asem)
            vector.sem_clear(usem)
            # Zero the writeback column (partitions 1..127 are never
            # written by compute); same-engine order puts this before the
            # partition-0 accumulator write of the reduce.
            vector.memset(col[:, :], 0.0)
            # u = (mu * mu) * C2^2.  The scalar-ptr operand of the g op is
            # fetched at dispatch, not engine-exec, so the in-order engine
            # does NOT cover the u hazard: round-trip a self-sem (its
            # propagation hides under the v op).
            vector.tensor_scalar(u[:, :], mu, mu, imm["C2SQ"],
                                 op0=ALU.mult,
                                 op1=ALU.mult)._wait_ge(dsem, 16).then_inc(
                                     usem, 1)
            # v = mu * LNPX2  (linearised e*ln(p2/p1) [+ ln p2 term])
            vector.tensor_scalar(v[:, :], mu, imm["LNPX2"], None,
                                 op0=ALU.mult)
            vector.wait_ge(usem, 1)
            # g = (P1n * u) + P0n   == -(P0 + u*P1)
            vector.scalar_tensor_tensor(g[:, :], p1n, u[:, 0:1], p0n,
                                        op0=ALU.mult,
                                        op1=ALU.add).then_inc(esem, 1)
            # acc = sum(pw * w)
            vector.scalar_tensor_tensor(
                tm[:, :], pw[:, :], 1.0, wt, op0=ALU.mult, op1=ALU.mult,
                accum_out=acc)._wait_ge(asem, 1).then_inc(vsem, 1)

        @block.scalar
        def _(scalar):
            scalar.sem_clear(esem)
            # pw = exp(g + v)
            scalar.activation(pw[:, :], g[:, :], AF.Exp, bias=v[:, 0:1],
                              scale=1.0)._wait_ge(esem, 1).then_inc(asem, 1)

        @block.gpsimd
        def _(gpsimd):
            gpsimd.sem_clear(vsem)
            gpsimd.sem_clear(psem)
            # kv_writeback ucode lives in the 'attn' Q7 library.
            gpsimd.load_library(library_config.attn)
            gpsimd.sem_clear(csem)
            gpsimd.memset(cidx[:, :], 0).then_inc(csem, 1)
            gpsimd.wait_ge(csem, 1)
            # Pre-generate the writeback descriptor during the input-DMA
            # dead time; the trigger fires it once the accumulator is ready.
            out4d = bass.AP(out, 0, [[128, 1], [1, 128], [1, 1], [1, 1]])
            in4d = bass.AP(col, 0, [[1, 128], [1, 1], [1, 1], [1, 1]])
            gpsimd.kv_writeback(out4d, in4d, cidx[:, :], prepare_only=True,
                                sem=odsem).then_inc(psem, 1)
            gpsimd.wait_ge(psem, 1)
            gpsimd.wait_ge(vsem, 1)
            gpsimd.trigger_dma(count=1)

    _strip_init_preamble(nc, pre_names)
    # Raw Bass skips Bacc's extended-inst codegen pass; without it the NEFF
    # compiler sees empty .instr bytes for kv_writeback/trigger/lib-reload
    # ("ISA wrong length").
    mybir.codegen_inst_isa_subclasses(nc)
    _split_multiwaits(nc)
    return nc, rows, post


def _in_maps(mu_val, rows):
    maps = []
    for r in rows:
        xin = np.empty((1, 1 + len(r)), dtype=np.float32)
        xin[0, 0] = mu_val
        xin[0, 1:] = r
        maps.append({"xin": xin})
    return maps


def build_program(T: int, s: int):
    """The program actually run/timed for inputs (T, s); handles the s==0
    remap.  Returns (nc, rows, post) or None if the answer is closed-form."""
    if T == 0:
        return None
    if s == 0:
        if T == 1:
            return None
        return _build_program(T - 1, 1, True)
    return _build_program(T, s, False)


def kernel(mu: np.ndarray, idx_T, idx_s) -> np.ndarray:
    T = int(idx_T)
    s = int(idx_s)
    mu_val = np.float32(np.asarray(mu).reshape(-1)[0])

    if T == 0:
        # A^0 = I
        return np.array([[1.0 if s == IDX_Z else 0.0]], dtype=np.float32)
    if s == 0 and T == 1:
        return np.array([[0.0]], dtype=np.float32)  # row IDX_Z sees nothing

    nc, rows, post = build_program(T, s)
    results = run_bass_kernel_spmd(nc, _in_maps(mu_val, rows),
                                   list(range(N_CORES))).results
    total = math.fsum(float(results[c]["out"][0, 0]) for c in range(N_CORES))
    return np.array([[total * post]], dtype=np.float32)


if __name__ == "__main__":
    out = kernel(np.array([-1.3152148], dtype=np.float32), 10000, 256)
    print("kernel output:", out)


# revision 24
# speedup vs baseline: 1.6717x; 1.0873x over previous
"""Trainium2 Bass kernel for nn_MCModel_84559316123793.

The reference iterates w <- A @ w idx_T times (tridiagonal transition
matrix, absorbing boundaries) and returns (A^T)[IDX_Z, idx_s].  With the
start index interior, the dynamics live in the (NX-1)-dim tridiagonal
Toeplitz matrix B = tridiag(p2, pmid, p1) whose eigensystem is the
discrete sine transform:

  (B^T)[z,s] = (2/NX) * (p2/p1)^((z-s)/2)
               * sum_k lam_k^T sin(z k pi/NX) sin(s k pi/NX),
  lam_k = pmid + 2 sqrt(p1 p2) cos(k pi/NX).

z = 512 makes sin(z k pi/NX) vanish for even k, so only odd modes carry
weight, and for T >~ 1000 only the few dozen lowest survive exp
underflow; the host picks the survivors (mu-independently, from T alone)
and shards them across the 8 cores.

All mu-dependence reduces, to O(u^2) accuracy (u = (mu*DT/DX)^2 <~ 3e-5),
to two scalars the device derives from the DMA'd mu:

  u = (mu*C2)^2,   v = mu*LNPX2        (e*ln(p2/p1) linearised in mu)
  pe_k = v - P0_k - u*P1_k             (host tables: P0 = -T ln lam_k|_0,
                                        P1 = dP0/du, exact at u=0)
  out  = sum_k w_k exp(pe_k)           (w_k = signed DST weights)

Five engine instructions total (3 DVE + 1 ACT exp + 1 DVE reduce)
between an input DMA and an output DMA, in a raw-Bass Block (no Tile
prologue/epilogue).  The host sums the 8 per-core partials and applies
the mu-independent prefactor 2/NX.  Linearisation error is <~ 5e-5
relative for |mu| <= 5 (tolerance 2e-2).
"""

import math

import numpy as np

import concourse.bass as bass
import concourse.mybir as mybir
from concourse import library_config
from concourse.bass_utils import run_bass_kernel_spmd

# Model constants (fixed by the problem definition)
SIGMA = 1.0
A_DOM = 2.0
Z_POS = 1.0
DT = 2e-06
NX = 1024
DX = A_DOM / NX
IDX_Z = int(round(Z_POS / DX))  # 512

N_CORES = 8

F32 = mybir.dt.float32
AF = mybir.ActivationFunctionType
ALU = mybir.AluOpType

# Derived immediates
A0 = SIGMA * SIGMA * DT / (DX * DX)  # p1+p2 at mu=0 (= 0.524288)
C2 = DT / DX                         # x = mu*C2 = p1-p2
A1 = (2.0 * A0 - 1.0) / (2.0 * A0)   # d(2 sqrt(p1 p2))/du
TC = 1.0 / (2.0 * A0)                # d(p1+p2-2 sqrt(p1 p2))/du

# Modes whose T*|ln lam| exceeds this are exp-underflow dead for any
# plausible mu shift (|u*P1| + |v| << 20) and are dropped host-side.
P0_CUT = 120.0
P0_DEAD = 200.0  # pe = v - 200 -> exp == 0.0f


def _split_multiwaits(nc):
    """This container's walrus rejects instructions carrying more than one
    sem-wait ("Too many sync wait commands").  Hoist all but the last onto
    single-wait NOPs inserted just before the offender on the same engine."""
    for bb in nc.main_func.blocks:
        insts = list(bb.instructions)
        changed = False
        out = []
        for ins in insts:
            si = ins.sync_info
            if si is not None and len(si.on_wait) > 1:
                waits = list(si.on_wait)
                for w in waits[:-1]:
                    nop = mybir.InstNoOp(
                        name=f"{ins.name}-wsplit-{w.ant_name}", ins=[], outs=[])
                    nop.engine = ins.engine
                    nop.sync_info = mybir.SyncInfo(on_wait=[w], on_update=[])
                    out.append(nop)
                ins.sync_info = mybir.SyncInfo(
                    on_wait=[waits[-1]], on_update=list(si.on_update))
                changed = True
            out.append(ins)
        if changed:
            bb.instructions = out


def _mode_tables(T: int, s_eff: int, extra_p2: bool):
    """Per-core host tables (mu-independent; they bake only T, s, geometry).

    Returns (npc, per-core [P0n | P1n | W] rows, device immediates, post
    scale).  P0n/P1n are negated so the device computes g = -P0 - u*P1 with
    a single (P1n * u) + P0n op and pe = g + v inside the ACT exp bias."""
    k = np.arange(1, NX, 2, dtype=np.float64)  # odd modes only (z = 512)
    th = k * np.pi / NX
    om = 1.0 - np.cos(th)
    lam0 = 1.0 - A0 * om                       # lam at u=0 (exact)
    P0 = -T * np.log(np.abs(lam0))
    P1 = T * (TC + A1 * om) / lam0             # dP0/du, exact at u=0
    w = np.sin(IDX_Z * th) * np.sin(s_eff * th)
    if T % 2 == 1:
        w = w * np.sign(lam0)

    # s==0 factor p2 = (A0 - x + u)/2: ln p2 folds into the linear model as
    # ln(A0/2) - x/A0 + u*(1/A0 - 1/(2 A0^2)); the constant goes to the
    # host post-scale, the x term into LNPX2, the u term into P1.
    e_coef = 0.5 * (IDX_Z - s_eff)
    lnp_x = -2.0 * e_coef / A0                 # d(e*ln(p2/p1))/dx
    post = 2.0 / NX
    if extra_p2:
        lnp_x -= 1.0 / A0
        P1 = P1 - (1.0 / A0 - 1.0 / (2.0 * A0 * A0))
        post *= A0 / 2.0

    keep = P0 <= P0_CUT
    nkeep = int(np.count_nonzero(keep))
    npc = min(len(k) // N_CORES, max(2, -(-nkeep // N_CORES)))
    P0k, P1k, wk = P0[keep][: npc * N_CORES], P1[keep][: npc * N_CORES], \
        w[keep][: npc * N_CORES]

    rows = []
    for c in range(N_CORES):
        sl = slice(c * npc, (c + 1) * npc)
        p0r = np.full(npc, P0_DEAD)
        p1r = np.zeros(npc)
        wr = np.zeros(npc)
        n = len(P0k[sl])
        p0r[:n], p1r[:n], wr[:n] = P0k[sl], P1k[sl], wk[sl]
        rows.append(np.concatenate([-p0r, -p1r, wr]).astype(np.float32))

    imm = {"LNPX2": C2 * lnp_x, "C2SQ": C2 * C2}
    return npc, rows, imm, post


def _strip_init_preamble(nc, pre_names):
    """Remove Bass.__init__'s const-AP memsets and its all-engine barrier
    (~930ns before the input DMA can issue).  This program never reads the
    const APs, and its own semaphores carry all cross-engine ordering."""
    kill = (mybir.InstMemset, mybir.InstDrain, mybir.InstEventSemaphore)
    # SP/DVE/ACT preamble RegisterMoves only init the zero/bounds-check
    # registers, which nothing in this program's static-AP instructions
    # reads; dropping them lets the input DMA issue ~250ns earlier.  Pool's
    # are kept (Q7 ucode may consult them) -- Pool has dead time anyway.
    fast = (mybir.EngineType.SP, mybir.EngineType.DVE,
            mybir.EngineType.Activation)
    for bb in nc.main_func.blocks:
        bb.instructions = [
            ins for ins in bb.instructions
            if not (ins.name in pre_names and
                    (isinstance(ins, kill) or
                     (isinstance(ins, mybir.InstRegisterMove) and
                      ins.engine in fast)))
        ]


def _build_program(T: int, s_eff: int, extra_p2: bool):
    """Emit the SPMD per-core program: DMA in -> 5 ops -> triggered DMA out.

    The output rides a kv_writeback descriptor prepared on the Pool engine
    during the input-DMA dead time; once the reduce lands, trigger_dma
    fires it, so the post-compute output cost is ~50ns + transfer + the
    mandatory DMA-sem propagation instead of a full HWDGE round trip."""
    npc, rows, imm, post = _mode_tables(T, s_eff, extra_p2)
    nc = bass.Bass()
    pre_names = {ins.name for bb in nc.main_func.blocks
                 for ins in bb.instructions}

    xin = nc.declare_dram_parameter("xin", [1, 1 + 3 * npc], F32,
                                    isOutput=False)
    # kv_writeback scatters one value per partition: out[0, p] <- col[p].
    # Only out[0, 0] (partition 0 = the reduce accumulator) is meaningful.
    out = nc.declare_dram_parameter("out", [1, 128], F32, isOutput=True)

    with (
        nc.Block(no_gpsimd_drain=True) as block,
        nc.semaphore("dsem") as dsem,
        nc.semaphore("esem") as esem,
        nc.semaphore("asem") as asem,
        nc.semaphore("vsem") as vsem,
        nc.semaphore("usem") as usem,
        nc.semaphore("psem") as psem,
        nc.semaphore("csem") as csem,
        nc.semaphore("odsem") as odsem,
        nc.sbuf_tensor("x", [1, 1 + 3 * npc], F32) as x,
        nc.sbuf_tensor("v", [1, 1], F32) as v,
        nc.sbuf_tensor("u", [1, 1], F32) as u,
        nc.sbuf_tensor("g", [1, npc], F32) as g,
        nc.sbuf_tensor("pw", [1, npc], F32) as pw,
        nc.sbuf_tensor("tm", [1, npc], F32) as tm,
        nc.sbuf_tensor("col", [128, 1], F32) as col,
        nc.sbuf_tensor("cidx", [128, 1], mybir.dt.int32) as cidx,
    ):
        mu = x[:, 0:1]
        p0n = x[:, 1:1 + npc]
        p1n = x[:, 1 + npc:1 + 2 * npc]
        wt = x[:, 1 + 2 * npc:1 + 3 * npc]
        acc = col[0:1, 0:1]

        # Semaphores are NOT zeroed on allocation, and earlier NEFFs on the
        # same device leave residue.  Each consumer clears the sems it waits
        # on at stream start; every producer's first inc is >2us later (the
        # input-DMA latency), so clear-before-inc holds by construction.
        @block.sync
        def _(sync):
            sync.dma_start(x[:, :], xin[:, :]).then_inc(dsem, 16)

        @block.vector
        def _(vector):
            vector.sem_clear(dsem)
            vector.sem_clear(asem)
            vector.sem_clear(usem)
            # Zero the writeback column (partitions 1..127 are never
            # written by compute); same-engine order puts this before the
            # partition-0 accumulator write of the reduce.
            vector.memset(col[:, :], 0.0)
            # u = (mu * mu) * C2SQ; mu scalar-ptrs are fetched at dispatch,
            # which this op's own dsem wait precedes.
            vector.tensor_scalar(u[:, :], mu, mu, imm["C2SQ"], op0=ALU.mult,
                                 op1=ALU.mult)._wait_ge(dsem, 16).then_inc(
                                     usem, 1)
            # v = mu * LNPX2  (linearised e*ln(p2/p1) [+ ln p2 term]);
            # executes while the usem update is in flight, hiding it.
            vector.tensor_scalar(v[:, :], mu, imm["LNPX2"], None,
                                 op0=ALU.mult)
            # g = (P1n * u) + P0n == -(P0 + u*P1).  Every same-engine RAW
            # needs a sem on this deep-pipelined engine; the wait rides the
            # instruction so no extra decode slot is spent.
            vector.scalar_tensor_tensor(
                g[:, :], p1n, u[:, 0:1], p0n, op0=ALU.mult,
                op1=ALU.add)._wait_ge(usem, 1).then_inc(esem, 1)
            # acc = sum(pw * w)
            vector.scalar_tensor_tensor(
                tm[:, :], pw[:, :], 1.0, wt, op0=ALU.mult, op1=ALU.mult,
                accum_out=acc)._wait_ge(asem, 1).then_inc(vsem, 1)


        @block.scalar
        def _(scalar):
            scalar.sem_clear(esem)
            # pw = exp(g + v)
            scalar.activation(pw[:, :], g[:, :], AF.Exp, bias=v[:, 0:1],
                              scale=1.0)._wait_ge(esem, 1).then_inc(asem, 1)

        @block.gpsimd
        def _(gpsimd):
            gpsimd.sem_clear(vsem)
            gpsimd.sem_clear(psem)
            # kv_writeback ucode lives in the 'attn' Q7 library.
            gpsimd.load_library(library_config.attn)
            gpsimd.sem_clear(csem)
            gpsimd.memset(cidx[:, :], 0).then_inc(csem, 1)
            gpsimd.wait_ge(csem, 1)
            # Pre-generate the writeback descriptor during the input-DMA
            # dead time; the trigger fires it once the accumulator is ready.
            out4d = bass.AP(out, 0, [[128, 1], [1, 128], [1, 1], [1, 1]])
            in4d = bass.AP(col, 0, [[1, 128], [1, 1], [1, 1], [1, 1]])
            gpsimd.kv_writeback(out4d, in4d, cidx[:, :], prepare_only=True,
                                sem=odsem).then_inc(psem, 1)
            gpsimd.wait_ge(psem, 1)
            gpsimd.wait_ge(vsem, 1)
            gpsimd.trigger_dma(count=1)

    _strip_init_preamble(nc, pre_names)
    # Raw Bass skips Bacc's extended-inst codegen pass; without it the NEFF
    # compiler sees empty .instr bytes for kv_writeback/trigger/lib-reload
    # ("ISA wrong length").
    mybir.codegen_inst_isa_subclasses(nc)
    _split_multiwaits(nc)
    return nc, rows, post


def _in_maps(mu_val, rows):
    maps = []
    for r in rows:
        xin = np.empty((1, 1 + len(r)), dtype=np.float32)
        xin[0, 0] = mu_val
        xin[0, 1:] = r
        maps.append({"xin": xin})
    return maps


def build_program(T: int, s: int):
    """The program actually run/timed for inputs (T, s); handles the s==0
    remap.  Returns (nc, rows, post) or None if the answer is closed-form."""
    if T == 0:
        return None
    if s == 0:
        if T == 1:
            return None
        return _build_program(T - 1, 1, True)
    return _build_program(T, s, False)


def kernel(mu: np.ndarray, idx_T, idx_s) -> np.ndarray:
    T = int(idx_T)
    s = int(idx_s)
    mu_val = np.float32(np.asarray(mu).reshape(-1)[0])

    if T == 0:
        # A^0 = I
        return np.array([[1.0 if s == IDX_Z else 0.0]], dtype=np.float32)
    if s == 0 and T == 1:
        return np.array([[0.0]], dtype=np.float32)  # row IDX_Z sees nothing

    nc, rows, post = build_program(T, s)
    results = run_bass_kernel_spmd(nc, _in_maps(mu_val, rows),
                                   list(range(N_CORES))).results
    total = math.fsum(float(results[c]["out"][0, 0]) for c in range(N_CORES))
    return np.array([[total * post]], dtype=np.float32)


if __name__ == "__main__":
    out = kernel(np.array([-1.3152148], dtype=np.float32), 10000, 256)
    print("kernel output:", out)


# revision 25
# speedup vs baseline: 1.7194x; 1.0286x over previous
"""Trainium2 Bass kernel for nn_MCModel_84559316123793.

The reference iterates w <- A @ w idx_T times (tridiagonal transition
matrix, absorbing boundaries) and returns (A^T)[IDX_Z, idx_s].  With the
start index interior, the dynamics live in the (NX-1)-dim tridiagonal
Toeplitz matrix B = tridiag(p2, pmid, p1) whose eigensystem is the
discrete sine transform:

  (B^T)[z,s] = (2/NX) * (p2/p1)^((z-s)/2)
               * sum_k lam_k^T sin(z k pi/NX) sin(s k pi/NX),
  lam_k = pmid + 2 sqrt(p1 p2) cos(k pi/NX).

z = 512 makes sin(z k pi/NX) vanish for even k, so only odd modes carry
weight, and for T >~ 1000 only the few dozen lowest survive exp
underflow; the host picks the survivors (mu-independently, from T alone)
and shards them across the 8 cores.

All mu-dependence reduces, to O(u^2) accuracy (u = (mu*DT/DX)^2 <~ 3e-5),
to two scalars the device derives from the DMA'd mu:

  u = (mu*C2)^2,   v = mu*LNPX2        (e*ln(p2/p1) linearised in mu)
  pe_k = v - P0_k - u*P1_k             (host tables: P0 = -T ln lam_k|_0,
                                        P1 = dP0/du, exact at u=0)
  out  = sum_k w_k exp(pe_k)           (w_k = signed DST weights)

Five engine instructions total (3 DVE + 1 ACT exp + 1 DVE reduce)
between an input DMA and an output DMA, in a raw-Bass Block (no Tile
prologue/epilogue).  The host sums the 8 per-core partials and applies
the mu-independent prefactor 2/NX.  Linearisation error is <~ 5e-5
relative for |mu| <= 5 (tolerance 2e-2).
"""

import math

import numpy as np

import concourse.bass as bass
import concourse.mybir as mybir
from concourse import library_config
from concourse.bass_utils import run_bass_kernel_spmd

# Model constants (fixed by the problem definition)
SIGMA = 1.0
A_DOM = 2.0
Z_POS = 1.0
DT = 2e-06
NX = 1024
DX = A_DOM / NX
IDX_Z = int(round(Z_POS / DX))  # 512

N_CORES = 8

F32 = mybir.dt.float32
AF = mybir.ActivationFunctionType
ALU = mybir.AluOpType

# Derived immediates
A0 = SIGMA * SIGMA * DT / (DX * DX)  # p1+p2 at mu=0 (= 0.524288)
C2 = DT / DX                         # x = mu*C2 = p1-p2
A1 = (2.0 * A0 - 1.0) / (2.0 * A0)   # d(2 sqrt(p1 p2))/du
TC = 1.0 / (2.0 * A0)                # d(p1+p2-2 sqrt(p1 p2))/du

# Modes whose T*|ln lam| exceeds this are exp-underflow dead for any
# plausible mu shift (|u*P1| + |v| << 20) and are dropped host-side.
P0_CUT = 120.0
P0_DEAD = 200.0  # pe = v - 200 -> exp == 0.0f


def _split_multiwaits(nc):
    """This container's walrus rejects instructions carrying more than one
    sem-wait ("Too many sync wait commands").  Hoist all but the last onto
    single-wait NOPs inserted just before the offender on the same engine."""
    for bb in nc.main_func.blocks:
        insts = list(bb.instructions)
        changed = False
        out = []
        for ins in insts:
            si = ins.sync_info
            if si is not None and len(si.on_wait) > 1:
                waits = list(si.on_wait)
                for w in waits[:-1]:
                    nop = mybir.InstNoOp(
                        name=f"{ins.name}-wsplit-{w.ant_name}", ins=[], outs=[])
                    nop.engine = ins.engine
                    nop.sync_info = mybir.SyncInfo(on_wait=[w], on_update=[])
                    out.append(nop)
                ins.sync_info = mybir.SyncInfo(
                    on_wait=[waits[-1]], on_update=list(si.on_update))
                changed = True
            out.append(ins)
        if changed:
            bb.instructions = out


def _mode_tables(T: int, s_eff: int, extra_p2: bool):
    """Per-core host tables (mu-independent; they bake only T, s, geometry).

    Returns (npc, per-core [P0n | P1n | W] rows, device immediates, post
    scale).  P0n/P1n are negated so the device computes g = -P0 - u*P1 with
    a single (P1n * u) + P0n op and pe = g + v inside the ACT exp bias."""
    k = np.arange(1, NX, 2, dtype=np.float64)  # odd modes only (z = 512)
    th = k * np.pi / NX
    om = 1.0 - np.cos(th)
    lam0 = 1.0 - A0 * om                       # lam at u=0 (exact)
    P0 = -T * np.log(np.abs(lam0))
    P1 = T * (TC + A1 * om) / lam0             # dP0/du, exact at u=0
    w = np.sin(IDX_Z * th) * np.sin(s_eff * th)
    if T % 2 == 1:
        w = w * np.sign(lam0)

    # s==0 factor p2 = (A0 - x + u)/2: ln p2 folds into the linear model as
    # ln(A0/2) - x/A0 + u*(1/A0 - 1/(2 A0^2)); the constant goes to the
    # host post-scale, the x term into LNPX2, the u term into P1.
    e_coef = 0.5 * (IDX_Z - s_eff)
    lnp_x = -2.0 * e_coef / A0                 # d(e*ln(p2/p1))/dx
    post = 2.0 / NX
    if extra_p2:
        lnp_x -= 1.0 / A0
        P1 = P1 - (1.0 / A0 - 1.0 / (2.0 * A0 * A0))
        post *= A0 / 2.0

    keep = P0 <= P0_CUT
    nkeep = int(np.count_nonzero(keep))
    npc = min(len(k) // N_CORES, max(2, -(-nkeep // N_CORES)))
    P0k, P1k, wk = P0[keep][: npc * N_CORES], P1[keep][: npc * N_CORES], \
        w[keep][: npc * N_CORES]

    rows = []
    for c in range(N_CORES):
        sl = slice(c * npc, (c + 1) * npc)
        p0r = np.full(npc, P0_DEAD)
        p1r = np.zeros(npc)
        wr = np.zeros(npc)
        n = len(P0k[sl])
        p0r[:n], p1r[:n], wr[:n] = P0k[sl], P1k[sl], wk[sl]
        rows.append(np.concatenate([-p0r, -p1r, wr]).astype(np.float32))

    imm = {"LNPX2": C2 * lnp_x, "C2SQ": C2 * C2}
    return npc, rows, imm, post


def _strip_init_preamble(nc, pre_names):
    """Remove Bass.__init__'s const-AP memsets and its all-engine barrier
    (~930ns before the input DMA can issue).  This program never reads the
    const APs, and its own semaphores carry all cross-engine ordering."""
    kill = (mybir.InstMemset, mybir.InstDrain, mybir.InstEventSemaphore)
    # SP/DVE/ACT preamble RegisterMoves only init the zero/bounds-check
    # registers, which nothing in this program's static-AP instructions
    # reads; dropping them lets the input DMA issue ~250ns earlier.  Pool's
    # are kept (Q7 ucode may consult them) -- Pool has dead time anyway.
    fast = (mybir.EngineType.SP, mybir.EngineType.DVE,
            mybir.EngineType.Activation)
    for bb in nc.main_func.blocks:
        bb.instructions = [
            ins for ins in bb.instructions
            if not (ins.name in pre_names and
                    (isinstance(ins, kill) or
                     (isinstance(ins, mybir.InstRegisterMove) and
                      ins.engine in fast)))
        ]


def _build_program(T: int, s_eff: int, extra_p2: bool):
    """Emit the SPMD per-core program: DMA in -> 5 ops -> triggered DMA out.

    The output rides a kv_writeback descriptor prepared on the Pool engine
    during the input-DMA dead time; once the reduce lands, trigger_dma
    fires it, so the post-compute output cost is ~50ns + transfer + the
    mandatory DMA-sem propagation instead of a full HWDGE round trip."""
    npc, rows, imm, post = _mode_tables(T, s_eff, extra_p2)
    nc = bass.Bass()
    pre_names = {ins.name for bb in nc.main_func.blocks
                 for ins in bb.instructions}

    xin = nc.declare_dram_parameter("xin", [1, 1 + 3 * npc], F32,
                                    isOutput=False)
    # kv_writeback scatters one value per partition: out[0, p] <- col[p].
    # Only out[0, 0] (partition 0 = the reduce accumulator) is meaningful.
    out = nc.declare_dram_parameter("out", [1, 128], F32, isOutput=True)

    with (
        nc.Block(no_gpsimd_drain=True) as block,
        nc.semaphore("dsem") as dsem,
        nc.semaphore("esem") as esem,
        nc.semaphore("asem") as asem,
        nc.semaphore("vsem") as vsem,
        nc.semaphore("usem") as usem,
        nc.semaphore("psem") as psem,
        nc.semaphore("csem") as csem,
        nc.semaphore("odsem") as odsem,
        nc.sbuf_tensor("x", [1, 1 + 3 * npc], F32) as x,
        nc.sbuf_tensor("v", [1, 1], F32) as v,
        nc.sbuf_tensor("u", [1, 1], F32) as u,
        nc.sbuf_tensor("g", [1, npc], F32) as g,
        nc.sbuf_tensor("pw", [1, npc], F32) as pw,
        nc.sbuf_tensor("tm", [1, npc], F32) as tm,
        nc.sbuf_tensor("col", [128, 1], F32) as col,
        nc.sbuf_tensor("cidx", [128, 1], mybir.dt.int32) as cidx,
    ):
        mu = x[:, 0:1]
        p0n = x[:, 1:1 + npc]
        p1n = x[:, 1 + npc:1 + 2 * npc]
        wt = x[:, 1 + 2 * npc:1 + 3 * npc]
        acc = col[0:1, 0:1]

        # Semaphores are NOT zeroed on allocation, and earlier NEFFs on the
        # same device leave residue.  Each consumer clears the sems it waits
        # on at stream start; every producer's first inc is >2us later (the
        # input-DMA latency), so clear-before-inc holds by construction.
        #
        # The input DMA is emitted in the entry block, before the per-engine
        # body branches, so SP issues it at ~t=25 instead of after a branch.
        nc.sync.dma_start(x[:, :], xin[:, :]).then_inc(dsem, 16)

        @block.vector
        def _(vector):
            vector.sem_clear(dsem)
            vector.sem_clear(asem)
            vector.sem_clear(usem)
            # Zero the writeback column (partitions 1..127 are never
            # written by compute); same-engine order puts this before the
            # partition-0 accumulator write of the reduce.
            vector.memset(col[:, :], 0.0)
            # u = (mu * mu) * C2SQ; mu scalar-ptrs are fetched at dispatch,
            # which this op's own dsem wait precedes.
            vector.tensor_scalar(u[:, :], mu, mu, imm["C2SQ"], op0=ALU.mult,
                                 op1=ALU.mult)._wait_ge(dsem, 16).then_inc(
                                     usem, 1)
            # v = mu * LNPX2  (linearised e*ln(p2/p1) [+ ln p2 term]);
            # executes while the usem update is in flight, hiding it.
            vector.tensor_scalar(v[:, :], mu, imm["LNPX2"], None,
                                 op0=ALU.mult)
            # g = (P1n * u) + P0n == -(P0 + u*P1).  Every same-engine RAW
            # needs a sem on this deep-pipelined engine; the wait rides the
            # instruction so no extra decode slot is spent.
            vector.scalar_tensor_tensor(
                g[:, :], p1n, u[:, 0:1], p0n, op0=ALU.mult,
                op1=ALU.add)._wait_ge(usem, 1).then_inc(esem, 1)
            # acc = sum(pw * w)
            vector.scalar_tensor_tensor(
                tm[:, :], pw[:, :], 1.0, wt, op0=ALU.mult, op1=ALU.mult,
                accum_out=acc)._wait_ge(asem, 1).then_inc(vsem, 1)


        @block.scalar
        def _(scalar):
            scalar.sem_clear(esem)
            # pw = exp(g + v)
            scalar.activation(pw[:, :], g[:, :], AF.Exp, bias=v[:, 0:1],
                              scale=1.0)._wait_ge(esem, 1).then_inc(asem, 1)

        @block.gpsimd
        def _(gpsimd):
            gpsimd.sem_clear(vsem)
            gpsimd.sem_clear(psem)
            # kv_writeback ucode lives in the 'attn' Q7 library.
            gpsimd.load_library(library_config.attn)
            gpsimd.sem_clear(csem)
            gpsimd.memset(cidx[:, :], 0).then_inc(csem, 1)
            gpsimd.wait_ge(csem, 1)
            # Pre-generate the writeback descriptor during the input-DMA
            # dead time; the trigger fires it once the accumulator is ready.
            out4d = bass.AP(out, 0, [[128, 1], [1, 128], [1, 1], [1, 1]])
            in4d = bass.AP(col, 0, [[1, 128], [1, 1], [1, 1], [1, 1]])
            gpsimd.kv_writeback(out4d, in4d, cidx[:, :], prepare_only=True,
                                sem=odsem).then_inc(psem, 1)
            gpsimd.wait_ge(psem, 1)
            gpsimd.trigger_dma(count=1)._wait_ge(vsem, 1)

    _strip_init_preamble(nc, pre_names)
    # Raw Bass skips Bacc's extended-inst codegen pass; without it the NEFF
    # compiler sees empty .instr bytes for kv_writeback/trigger/lib-reload
    # ("ISA wrong length").
    mybir.codegen_inst_isa_subclasses(nc)
    _split_multiwaits(nc)
    return nc, rows, post


def _in_maps(mu_val, rows):
    maps = []
    for r in rows:
        xin = np.empty((1, 1 + len(r)), dtype=np.float32)
        xin[0, 0] = mu_val
        xin[0, 1:] = r
        maps.append({"xin": xin})
    return maps


def build_program(T: int, s: int):
    """The program actually run/timed for inputs (T, s); handles the s==0
    remap.  Returns (nc, rows, post) or None if the answer is closed-form."""
    if T == 0:
        return None
    if s == 0:
        if T == 1:
            return None
        return _build_program(T - 1, 1, True)
    return _build_program(T, s, False)


def kernel(mu: np.ndarray, idx_T, idx_s) -> np.ndarray:
    T = int(idx_T)
    s = int(idx_s)
    mu_val = np.float32(np.asarray(mu).reshape(-1)[0])

    if T == 0:
        # A^0 = I
        return np.array([[1.0 if s == IDX_Z else 0.0]], dtype=np.float32)
    if s == 0 and T == 1:
        return np.array([[0.0]], dtype=np.float32)  # row IDX_Z sees nothing

    nc, rows, post = build_program(T, s)
    results = run_bass_kernel_spmd(nc, _in_maps(mu_val, rows),
                                   list(range(N_CORES))).results
    total = math.fsum(float(results[c]["out"][0, 0]) for c in range(N_CORES))
    return np.array([[total * post]], dtype=np.float32)


if __name__ == "__main__":
    out = kernel(np.array([-1.3152148], dtype=np.float32), 10000, 256)
    print("kernel output:", out)


# revision 26
# speedup vs baseline: 1.7947x; 1.0438x over previous
"""Trainium2 Bass kernel for nn_MCModel_84559316123793.

The reference iterates w <- A @ w idx_T times (tridiagonal transition
matrix, absorbing boundaries) and returns (A^T)[IDX_Z, idx_s].  With the
start index interior, the dynamics live in the (NX-1)-dim tridiagonal
Toeplitz matrix B = tridiag(p2, pmid, p1) whose eigensystem is the
discrete sine transform:

  (B^T)[z,s] = (2/NX) * (p2/p1)^((z-s)/2)
               * sum_k lam_k^T sin(z k pi/NX) sin(s k pi/NX),
  lam_k = pmid + 2 sqrt(p1 p2) cos(k pi/NX).

z = 512 makes sin(z k pi/NX) vanish for even k, so only odd modes carry
weight, and for T >~ 1000 only the few dozen lowest survive exp
underflow; the host picks the survivors (mu-independently, from T alone)
and shards them across the 8 cores.

All mu-dependence reduces, to O(u^2) accuracy (u = (mu*DT/DX)^2 <~ 3e-5),
to two scalars the device derives from the DMA'd mu:

  u = (mu*C2)^2,   v = mu*LNPX2        (e*ln(p2/p1) linearised in mu)
  pe_k = v - P0_k - u*P1_k             (host tables: P0 = -T ln lam_k|_0,
                                        P1 = dP0/du, exact at u=0)
  out  = sum_k w_k exp(pe_k)           (w_k = signed DST weights)

Five engine instructions total (3 DVE + 1 ACT exp + 1 DVE reduce)
between an input DMA and an output DMA, in a raw-Bass Block (no Tile
prologue/epilogue).  The host sums the 8 per-core partials and applies
the mu-independent prefactor 2/NX.  Linearisation error is <~ 5e-5
relative for |mu| <= 5 (tolerance 2e-2).
"""

import math

import numpy as np

import concourse.bass as bass
import concourse.mybir as mybir
from concourse import library_config
from concourse.bass_utils import run_bass_kernel_spmd

# Model constants (fixed by the problem definition)
SIGMA = 1.0
A_DOM = 2.0
Z_POS = 1.0
DT = 2e-06
NX = 1024
DX = A_DOM / NX
IDX_Z = int(round(Z_POS / DX))  # 512

N_CORES = 8

F32 = mybir.dt.float32
AF = mybir.ActivationFunctionType
ALU = mybir.AluOpType

# Derived immediates
A0 = SIGMA * SIGMA * DT / (DX * DX)  # p1+p2 at mu=0 (= 0.524288)
C2 = DT / DX                         # x = mu*C2 = p1-p2
A1 = (2.0 * A0 - 1.0) / (2.0 * A0)   # d(2 sqrt(p1 p2))/du
TC = 1.0 / (2.0 * A0)                # d(p1+p2-2 sqrt(p1 p2))/du

# Modes whose T*|ln lam| exceeds this are exp-underflow dead for any
# plausible mu shift (|u*P1| + |v| << 20) and are dropped host-side.
P0_CUT = 120.0
P0_DEAD = 200.0  # pe = v - 200 -> exp == 0.0f


def _split_multiwaits(nc):
    """This container's walrus rejects instructions carrying more than one
    sem-wait ("Too many sync wait commands").  Hoist all but the last onto
    single-wait NOPs inserted just before the offender on the same engine."""
    for bb in nc.main_func.blocks:
        insts = list(bb.instructions)
        changed = False
        out = []
        for ins in insts:
            si = ins.sync_info
            if si is not None and len(si.on_wait) > 1:
                waits = list(si.on_wait)
                for w in waits[:-1]:
                    nop = mybir.InstNoOp(
                        name=f"{ins.name}-wsplit-{w.ant_name}", ins=[], outs=[])
                    nop.engine = ins.engine
                    nop.sync_info = mybir.SyncInfo(on_wait=[w], on_update=[])
                    out.append(nop)
                ins.sync_info = mybir.SyncInfo(
                    on_wait=[waits[-1]], on_update=list(si.on_update))
                changed = True
            out.append(ins)
        if changed:
            bb.instructions = out


def _mode_tables(T: int, s_eff: int, extra_p2: bool):
    """Per-core host tables (mu-independent; they bake only T, s, geometry).

    Returns (npc, per-core [P0n | P1n | W] rows, device immediates, post
    scale).  P0n/P1n are negated so the device computes g = -P0 - u*P1 with
    a single (P1n * u) + P0n op and pe = g + v inside the ACT exp bias."""
    k = np.arange(1, NX, 2, dtype=np.float64)  # odd modes only (z = 512)
    th = k * np.pi / NX
    om = 1.0 - np.cos(th)
    lam0 = 1.0 - A0 * om                       # lam at u=0 (exact)
    P0 = -T * np.log(np.abs(lam0))
    P1 = T * (TC + A1 * om) / lam0             # dP0/du, exact at u=0
    w = np.sin(IDX_Z * th) * np.sin(s_eff * th)
    if T % 2 == 1:
        w = w * np.sign(lam0)

    # s==0 factor p2 = (A0 - x + u)/2: ln p2 folds into the linear model as
    # ln(A0/2) - x/A0 + u*(1/A0 - 1/(2 A0^2)); the constant goes to the
    # host post-scale, the x term into LNPX2, the u term into P1.
    e_coef = 0.5 * (IDX_Z - s_eff)
    lnp_x = -2.0 * e_coef / A0                 # d(e*ln(p2/p1))/dx
    post = 2.0 / NX
    if extra_p2:
        lnp_x -= 1.0 / A0
        P1 = P1 - (1.0 / A0 - 1.0 / (2.0 * A0 * A0))
        post *= A0 / 2.0

    # Fold exp(-P0) into the weights host-side: the device computes only
    # pw = exp(u*P1C + v) (one ACT op via scale/bias pointers, no g op) and
    # sum(Wp * pw).  Dead modes simply get Wp = 0.
    keep = P0 <= P0_CUT
    nkeep = int(np.count_nonzero(keep))
    npc = min(len(k) // N_CORES, max(2, -(-nkeep // N_CORES)))
    p1c_all = (-P1 * (C2 * C2))[keep][: npc * N_CORES]
    wp_all = (w * np.exp(-P0))[keep][: npc * N_CORES]

    rows = []
    for c in range(N_CORES):
        sl = slice(c * npc, (c + 1) * npc)
        p1r = np.zeros(npc)
        wr = np.zeros(npc)
        n = len(p1c_all[sl])
        p1r[:n], wr[:n] = p1c_all[sl], wp_all[sl]
        rows.append(np.concatenate([p1r, wr]).astype(np.float32))

    imm = {"LNPX2": C2 * lnp_x}
    return npc, rows, imm, post


def _strip_init_preamble(nc, pre_names):
    """Remove Bass.__init__'s const-AP memsets and its all-engine barrier
    (~930ns before the input DMA can issue).  This program never reads the
    const APs, and its own semaphores carry all cross-engine ordering."""
    kill = (mybir.InstMemset, mybir.InstDrain, mybir.InstEventSemaphore)
    # SP/DVE/ACT preamble RegisterMoves only init the zero/bounds-check
    # registers, which nothing in this program's static-AP instructions
    # reads; dropping them lets the input DMA issue ~250ns earlier.  Pool's
    # are kept (Q7 ucode may consult them) -- Pool has dead time anyway.
    fast = (mybir.EngineType.SP, mybir.EngineType.DVE,
            mybir.EngineType.Activation)
    for bb in nc.main_func.blocks:
        bb.instructions = [
            ins for ins in bb.instructions
            if not (ins.name in pre_names and
                    (isinstance(ins, kill) or
                     (isinstance(ins, mybir.InstRegisterMove) and
                      ins.engine in fast)))
        ]


def _build_program(T: int, s_eff: int, extra_p2: bool):
    """Emit the SPMD per-core program: DMA in -> 5 ops -> triggered DMA out.

    The output rides a kv_writeback descriptor prepared on the Pool engine
    during the input-DMA dead time; once the reduce lands, trigger_dma
    fires it, so the post-compute output cost is ~50ns + transfer + the
    mandatory DMA-sem propagation instead of a full HWDGE round trip."""
    npc, rows, imm, post = _mode_tables(T, s_eff, extra_p2)
    nc = bass.Bass()
    pre_names = {ins.name for bb in nc.main_func.blocks
                 for ins in bb.instructions}

    xin = nc.declare_dram_parameter("xin", [1, 1 + 2 * npc], F32,
                                    isOutput=False)
    # kv_writeback scatters one value per partition: out[0, p] <- col[p].
    # Only out[0, 0] (partition 0 = the reduce accumulator) is meaningful.
    out = nc.declare_dram_parameter("out", [1, 128], F32, isOutput=True)

    with (
        nc.Block(no_gpsimd_drain=True) as block,
        nc.semaphore("dsem") as dsem,
        nc.semaphore("esem") as esem,
        nc.semaphore("asem") as asem,
        nc.semaphore("vsem") as vsem,
        nc.semaphore("psem") as psem,
        nc.semaphore("csem") as csem,
        nc.semaphore("odsem") as odsem,
        nc.sbuf_tensor("x", [1, 1 + 2 * npc], F32) as x,
        nc.sbuf_tensor("v", [1, 1], F32) as v,
        nc.sbuf_tensor("u", [1, 1], F32) as u,
        nc.sbuf_tensor("pw", [1, npc], F32) as pw,
        nc.sbuf_tensor("tm", [1, npc], F32) as tm,
        nc.sbuf_tensor("col", [128, 1], F32) as col,
        nc.sbuf_tensor("cidx", [128, 1], mybir.dt.int32) as cidx,
    ):
        mu = x[:, 0:1]
        p1c = x[:, 1:1 + npc]
        wt = x[:, 1 + npc:1 + 2 * npc]
        acc = col[0:1, 0:1]

        # Semaphores are NOT zeroed on allocation, and earlier NEFFs on the
        # same device leave residue.  Each consumer clears the sems it waits
        # on at stream start; every producer's first inc is >2us later (the
        # input-DMA latency), so clear-before-inc holds by construction.
        #
        # The input DMA is emitted in the entry block, before the per-engine
        # body branches, so SP issues it at ~t=25 instead of after a branch.
        nc.sync.dma_start(x[:, :], xin[:, :]).then_inc(dsem, 16)

        @block.vector
        def _(vector):
            vector.sem_clear(dsem)
            vector.sem_clear(asem)
            # Zero the writeback column (partitions 1..127 are never
            # written by compute); same-engine order puts this before the
            # partition-0 accumulator write of the reduce.
            vector.memset(col[:, :], 0.0)
            # u = mu * mu; the mu scalar-ptr is fetched at dispatch, which
            # this op's own dsem wait precedes.
            vector.tensor_scalar(u[:, :], mu, mu, None,
                                 op0=ALU.mult)._wait_ge(dsem, 16)
            # v = mu * LNPX2  (linearised e*ln(p2/p1) [+ ln p2 term]).
            # esem after v covers u too (same engine, in-order).
            vector.tensor_scalar(v[:, :], mu, imm["LNPX2"], None,
                                 op0=ALU.mult).then_inc(esem, 1)
            # acc = sum(pw * w)
            vector.scalar_tensor_tensor(
                tm[:, :], pw[:, :], 1.0, wt, op0=ALU.mult, op1=ALU.mult,
                accum_out=acc)._wait_ge(asem, 1).then_inc(vsem, 1)


        @block.scalar
        def _(scalar):
            scalar.sem_clear(esem)
            # pw = exp(P1C*u + v); u and v ride the scale/bias pointer
            # slots, so no per-element pre-add op is needed.
            scalar.activation(pw[:, :], p1c, AF.Exp, bias=v[:, 0:1],
                              scale=u[:, 0:1])._wait_ge(esem, 1).then_inc(
                                  asem, 1)

        @block.gpsimd
        def _(gpsimd):
            gpsimd.sem_clear(vsem)
            gpsimd.sem_clear(psem)
            # kv_writeback ucode lives in the 'attn' Q7 library.
            gpsimd.load_library(library_config.attn)
            gpsimd.sem_clear(csem)
            gpsimd.memset(cidx[:, :], 0).then_inc(csem, 1)
            gpsimd.wait_ge(csem, 1)
            # Pre-generate the writeback descriptor during the input-DMA
            # dead time; the trigger fires it once the accumulator is ready.
            out4d = bass.AP(out, 0, [[128, 1], [1, 128], [1, 1], [1, 1]])
            in4d = bass.AP(col, 0, [[1, 128], [1, 1], [1, 1], [1, 1]])
            gpsimd.kv_writeback(out4d, in4d, cidx[:, :], prepare_only=True,
                                sem=odsem).then_inc(psem, 1)
            gpsimd.wait_ge(psem, 1)
            gpsimd.trigger_dma(count=1)._wait_ge(vsem, 1)

    _strip_init_preamble(nc, pre_names)
    # Raw Bass skips Bacc's extended-inst codegen pass; without it the NEFF
    # compiler sees empty .instr bytes for kv_writeback/trigger/lib-reload
    # ("ISA wrong length").
    mybir.codegen_inst_isa_subclasses(nc)
    _split_multiwaits(nc)
    return nc, rows, post


def _in_maps(mu_val, rows):
    maps = []
    for r in rows:
        xin = np.empty((1, 1 + len(r)), dtype=np.float32)
        xin[0, 0] = mu_val
        xin[0, 1:] = r
        maps.append({"xin": xin})
    return maps


def build_program(T: int, s: int):
    """The program actually run/timed for inputs (T, s); handles the s==0
    remap.  Returns (nc, rows, post) or None if the answer is closed-form."""
    if T == 0:
        return None
    if s == 0:
        if T == 1:
            return None
        return _build_program(T - 1, 1, True)
    return _build_program(T, s, False)


def kernel(mu: np.ndarray, idx_T, idx_s) -> np.ndarray:
    T = int(idx_T)
    s = int(idx_s)
    mu_val = np.float32(np.asarray(mu).reshape(-1)[0])

    if T == 0:
        # A^0 = I
        return np.array([[1.0 if s == IDX_Z else 0.0]], dtype=np.float32)
    if s == 0 and T == 1:
        return np.array([[0.0]], dtype=np.float32)  # row IDX_Z sees nothing

    nc, rows, post = build_program(T, s)
    results = run_bass_kernel_spmd(nc, _in_maps(mu_val, rows),
                                   list(range(N_CORES))).results
    total = math.fsum(float(results[c]["out"][0, 0]) for c in range(N_CORES))
    return np.array([[total * post]], dtype=np.float32)


if __name__ == "__main__":
    out = kernel(np.array([-1.3152148], dtype=np.float32), 10000, 256)
    print("kernel output:", out)
